# revision 10
# baseline (speedup 1.0000x reference)
"""Trainium2 Bass kernel for capsule dynamic routing (nn_Capsule).

Reference computation:
    hat = (x @ kernel).reshape(B, N, C, D).transpose(0, 2, 1, 3)   # [B,C,N,D]
    b = 0; 3 routing iterations of:
        w = softmax(b, axis=capsules)
        o = squash(einsum('bcn,bcnd->bcd', w, hat))
        b += einsum('bcd,bcnd->bcn', o, hat)

Key reformulation (hat is never materialized):
    o[c,d]  = sum_i xw[c,i] * K[i,(c,d)]      xw = w[c,:] @ x      (A-op)
    bupd[c,n] = sum_i x[n,i] * oK[c,i]        oK[c,i] = sum_d o[c,d]*K[i,(c,d)]
This reduces 34 GFLOP of hat-work to ~100 MFLOP of small matmuls whose cost
is streaming K through the PE as stationary operands (bf16, fp32 accumulate).

Sharding: data-parallel over batch B=16 across 8 cores (2 items/core, fused
into the same matmuls via a 2-wide moving operand). kernel K replicated.

Precision/speed split: the four routing K-passes (A0, oK0, A1, oK1) only
perturb softmax logits, so their stationary K operands are fp8 e3m4 (4-bit
mantissa, host-prescaled by S8 to fill the e3m4 range) — fp8 weight loads
run 2x faster through the PE (FWL loads 4 fp8 cols/cycle vs 2 bf16). The
final A2 pass feeds the output directly and keeps a bf16 K copy. The 1/S8^2
unscale folds into the squash-scale activations (Square scale=1/S8, Exp
bias=-2*ln(S8)), costing zero extra instructions.
"""

import math
import numpy as np
import ml_dtypes
from contextlib import ExitStack

NCORES = 8
B, N, DI = 16, 512, 256         # batch, input capsules, input dim
C, D = 64, 128                  # output capsules, capsule dim
NB = B // NCORES                # batch items per core
P = 128                         # SBUF partitions
NCH = N // P                    # 4 n-chunks
ICH = DI // P                   # 2 i-chunks
ROUTINGS = 3
EPS = 1e-7
S8 = 512.0                      # host pre-scale on K before e3m4 cast

_cache = {}


def _build_program(reps=0):
    """reps=0: plain single-shot program (graded path).
    reps>0: wrap the whole body (input DMA + compute + output DMA) in a
    For_i loop for wall-clock-difference benchmarking."""
    import concourse.bass_isa as bass_isa
    import concourse.mybir as mybir
    import concourse.tile as tile
    from concourse import bacc

    F32 = mybir.dt.float32
    BF16 = mybir.dt.bfloat16
    FP8 = mybir.dt.float8e3
    AF = mybir.ActivationFunctionType
    AX = mybir.AxisListType

    class _OneActSetBacc(bacc.Bacc):
        """Every activation func used here (Square/Ln/Exp/Copy) lives in the
        'natural_log_exp_and_others' table set, but the default chooser picks
        per-func sets greedily and flip-flops (one ~1.3us LoadActFuncSet per
        switch, on the critical path). Mask the other sets so exactly one
        table load is emitted; indices are preserved so act_func_set_id still
        points at the real act_info.json entry."""

        def insert_act_table_loads(self):
            import bass_rust as _br
            from concourse.hw_specs import get_activation_tables

            has_activation = any(
                isinstance(i, mybir.InstActivation)
                for b in self.main_func.blocks
                for i in b.instructions
            )
            if not has_activation:
                return
            tables = [
                (name, funcs if name == "natural_log_exp_and_others" else set())
                for name, funcs in get_activation_tables(self.m.arch).items()
            ]
            _br.insert_act_table_loads(self, tables)

    nc = _OneActSetBacc("TRN2", target_bir_lowering=False, debug=False)

    x_d = nc.dram_tensor("x", [P, NB, NCH, DI], BF16, kind="ExternalInput").ap()
    xT_d = nc.dram_tensor("xT", [P, NB, ICH, N], BF16, kind="ExternalInput").ap()
    kk_d = nc.dram_tensor("kk", [P, ICH, C * D], FP8, kind="ExternalInput").ap()
    kt_d = nc.dram_tensor("kt", [P, C, DI], FP8, kind="ExternalInput").ap()
    kkb_d = nc.dram_tensor("kkb", [P, ICH, C * D], BF16, kind="ExternalInput").ap()
    xbar_d = nc.dram_tensor("xbar", [P, ICH, NB], BF16, kind="ExternalInput").ap()
    # output is [d, (c,b)]; the host does the final transpose to [b, c, d]
    out_d = nc.dram_tensor("out", [P, C * NB], F32, kind="ExternalOutput").ap()

    with tile.TileContext(nc) as tc, ExitStack() as ctx:
        big = ctx.enter_context(tc.tile_pool(name="big", bufs=1))
        wk = ctx.enter_context(tc.tile_pool(name="wk", bufs=2))
        psA = ctx.enter_context(tc.tile_pool(name="psA", bufs=2, space="PSUM"))
        psk = ctx.enter_context(tc.tile_pool(name="psk", bufs=2, space="PSUM"))
        pss = ctx.enter_context(tc.tile_pool(name="pss", bufs=3, space="PSUM"))

        kk = big.tile([P, ICH, C * D], FP8)
        kt = big.tile([P, C, DI], FP8)
        kkb = big.tile([P, ICH, C * D], BF16)
        xs = big.tile([P, NB, NCH, DI], BF16)
        xT = big.tile([P, NB, ICH, N], BF16)
        xbar = big.tile([P, ICH, NB], BF16)

        def emit_input_dmas():
            # DMA bandwidth is shared; order by first use: xbar + kk feed the
            # iteration-0 A-op, kt feeds b_op, xT feeds bupd, xs feeds xwT,
            # kkb feeds the final bf16 A2 pass. kk/kt/kkb are chunked so
            # capsule matmuls start while later chunks are still in flight.
            nc.sync.dma_start(out=xbar, in_=xbar_d)
            KCH = 16
            kw = (C * D) // KCH
            for j in range(KCH):
                nc.sync.dma_start(
                    out=kk[:, :, j * kw:(j + 1) * kw],
                    in_=kk_d[:, :, j * kw:(j + 1) * kw],
                )
            for j in range(KCH):
                cs = C // KCH
                nc.sync.dma_start(
                    out=kt[:, j * cs:(j + 1) * cs, :],
                    in_=kt_d[:, j * cs:(j + 1) * cs, :],
                )
            nc.sync.dma_start(out=xT, in_=xT_d)
            nc.sync.dma_start(out=xs, in_=x_d)
            for j in range(KCH):
                nc.sync.dma_start(
                    out=kkb[:, :, j * kw:(j + 1) * kw],
                    in_=kkb_d[:, :, j * kw:(j + 1) * kw],
                )

        bT = big.tile([P, NB, NCH, C], F32)     # routing logits, [n, b, nch, c]
        o_bf = big.tile([P, C, NB], BF16)       # current (unscaled) o, [d, (c,b)]
        epst = big.tile([P, 1], F32)
        nc.vector.memset(epst, EPS)
        zerot = big.tile([P, 1], F32)
        nc.vector.memset(zerot, 0.0)
        m2ls = big.tile([P, 1], F32)            # -2*ln(S8): unscales fp8 passes
        nc.vector.memset(m2ls, -2.0 * math.log(S8))
        # Dummy activation up front so the one LoadActFuncSet (~1.3us) runs
        # during the initial DMA wait instead of on the critical path.
        warm = big.tile([P, 1], F32)
        nc.scalar.activation(out=warm, in_=zerot[:], func=AF.Exp, bias=zerot[:])

        def a_op(rhs_at, stat):
            """o_pre[d,(c,b)] = sum_i rhs[i,(c,b)] * K[i,(c,d)] per capsule.
            stat: kk (fp8, routing passes) or kkb (bf16, final pass)."""
            po = psA.tile([P, C, NB], F32, tag="po")
            for c in range(C):
                for t in range(ICH):
                    nc.tensor.matmul(
                        po[:, c, :],
                        lhsT=stat[:, t, c * D:(c + 1) * D],
                        rhs=rhs_at(t, c),
                        start=(t == 0),
                        stop=(t == ICH - 1),
                    )
            return po

        def squash_scale(po, scaled):
            """scale[c,b] = sqrt(s+eps)/(0.5+s+eps), s = sum_d o_pre[d,(c,b)]^2,
            computed in free layout [*, (c,b)] (identical rows) so it can be
            applied with free-dim broadcasts downstream. The squash scale
            commutes past the (linear) oK matmul, so the PE proceeds straight
            from the A-op into the oK matmuls while this runs on ACT/Pool/DVE.
            Ln/Exp/Square share one activation-table set (no reloads);
            sqrt(t) = exp(0.5*ln(t)).
            scaled=True: po holds S8*o_pre (fp8 pass). Square's scale=1/S8
            recovers the true S, and Exp's bias=-2*ln(S8) folds the 1/S8^2 the
            downstream oks eviction needs (oks = pk * f/S8^2 with pk scaled by
            S8^2). scaled=False (bf16 final pass): returns the true f."""
            po2 = po[:].rearrange("p c b -> p (c b)")
            sq = wk.tile([P, C * NB], F32, tag="sq")
            nc.scalar.activation(out=sq, in_=po2, func=AF.Square,
                                 scale=(1.0 / S8) if scaled else 1.0)
            S = wk.tile([P, C * NB], F32, tag="S")
            nc.gpsimd.partition_all_reduce(S, sq, P, bass_isa.ReduceOp.add)
            num = wk.tile([P, C * NB], F32, tag="num")
            nc.scalar.activation(out=num, in_=S, func=AF.Ln, bias=epst[:])
            nc.scalar.activation(out=num, in_=num, func=AF.Exp, scale=0.5,
                                 bias=m2ls[:] if scaled else zerot[:])
            den = wk.tile([P, C * NB], F32, tag="den")
            nc.vector.tensor_scalar_add(den, S, 0.5 + EPS)
            nc.vector.reciprocal(den, den)
            scale = wk.tile([P, C * NB], F32, tag="scalef")
            nc.vector.tensor_mul(scale, num, den)
            return scale

        def b_op(it, scale):
            # oK_pre[i,(c,b)] per i-tile: contraction over d on partitions.
            pk = psk.tile([P, ICH, C, NB], F32, tag="pk")
            for c in range(C):
                for t in range(ICH):
                    nc.tensor.matmul(
                        pk[:, t, c, :],
                        lhsT=kt[:, c, t * P:(t + 1) * P],
                        rhs=o_bf[:, c, :],
                        start=True,
                        stop=True,
                    )
            # apply the squash scale during the PSUM->SBUF eviction; split per
            # (b, i-tile) so each bupd matmul starts as soon as its slice lands
            oks = wk.tile([P, NB, ICH, C], BF16, tag="oks")
            sc3 = scale[:].rearrange("p (c b) -> p b c", b=NB)
            for b in range(NB):
                for t in range(ICH):
                    nc.vector.tensor_tensor(
                        oks[:, b, t, :], pk[:, t, :, b], sc3[:, b, :],
                        mybir.AluOpType.mult,
                    )
            # bupd[n,(c)] = sum_i x[n,i] oK[c,i]  (lhsT = xT tiles); all four
            # n-tiles of one batch item share a PSUM tile -> one eviction.
            for b in range(NB):
                pb = pss.tile([P, NCH, C], F32, tag="pb")
                for nt in range(NCH):
                    for t in range(ICH):
                        nc.tensor.matmul(
                            pb[:, nt, :],
                            lhsT=xT[:, b, t, nt * P:(nt + 1) * P],
                            rhs=oks[:, b, t, :],
                            start=(t == 0),
                            stop=(t == ICH - 1),
                        )
                if it == 0:
                    nc.vector.tensor_copy(out=bT[:, b], in_=pb)
                else:
                    nc.vector.tensor_add(out=bT[:, b], in0=bT[:, b], in1=pb)

        def softmax_xw():
            # softmax over capsules (innermost free axis of bT); values are
            # O(1) so the max-subtraction is unnecessary.
            e = wk.tile([P, NB, NCH, C], F32, tag="e")
            nc.scalar.activation(out=e, in_=bT[:], func=AF.Exp, bias=zerot[:])
            es = wk.tile([P, NB, NCH], F32, tag="es")
            nc.vector.reduce_sum(out=es, in_=e, axis=AX.X)
            nc.vector.reciprocal(es, es)
            w = wk.tile([P, NB, NCH, C], BF16, tag="w")
            nc.vector.tensor_tensor(
                w, e, es[:, :, :, None].to_broadcast((P, NB, NCH, C)),
                mybir.AluOpType.mult,
            )
            # xwT[i,(c,b)] = sum_n x[n,i] w[n,c]  (lhsT = x tiles); both
            # i-tiles of one batch item share a PSUM tile -> one eviction.
            xwT = wk.tile([P, ICH, C, NB], BF16, tag="xwT")
            for b in range(NB):
                px = pss.tile([P, ICH, C], F32, tag="pb")
                for t in range(ICH):
                    for ch in range(NCH):
                        nc.tensor.matmul(
                            px[:, t, :],
                            lhsT=xs[:, b, ch, t * P:(t + 1) * P],
                            rhs=w[:, b, ch, :],
                            start=(ch == 0),
                            stop=(ch == NCH - 1),
                        )
                nc.vector.tensor_copy(out=xwT[:, :, :, b], in_=px)
            return xwT

        def body():
            emit_input_dmas()
            po = a_op(lambda t, c: xbar[:, t, :], kk)
            for it in range(ROUTINGS - 1):
                nc.vector.tensor_copy(
                    out=o_bf[:].rearrange("p c b -> p (c b)"),
                    in_=po[:].rearrange("p c b -> p (c b)"),
                )
                scale = squash_scale(po, scaled=True)
                b_op(it, scale)
                xwT = softmax_xw()
                last = it == ROUTINGS - 2
                po = a_op(lambda t, c, _x=xwT: _x[:, t, c, :],
                          kkb if last else kk)
            # final squash: o = o_pre * scale, emitted as [d, (c,b)]
            scale = squash_scale(po, scaled=False)
            oout = wk.tile([P, C * NB], F32, tag="oout")
            nc.vector.tensor_mul(oout, po[:].rearrange("p c b -> p (c b)"), scale)
            nc.sync.dma_start(out=out_d, in_=oout)

        if reps:
            with tc.For_i(0, reps, 1, hint_engines=(mybir.EngineType.PE,)):
                body()
        else:
            body()

    nc.compile()
    return nc


def _prep_inputs(x, kernel):
    bf16 = ml_dtypes.bfloat16
    e3m4 = ml_dtypes.float8_e3m4
    # single consistent e3m4 quantization of S8*K, then rearranged into the
    # two stationary layouts so kk and kt hold identical values
    k8 = (kernel * np.float32(S8)).astype(e3m4)
    kk = np.ascontiguousarray(
        k8.reshape(ICH, P, C * D).transpose(1, 0, 2))
    kt = np.ascontiguousarray(
        k8.reshape(DI, C, D).transpose(2, 1, 0))
    kkb = np.ascontiguousarray(
        kernel.reshape(ICH, P, C * D).transpose(1, 0, 2)).astype(bf16)
    in_maps = []
    for s in range(NCORES):
        xc = x[s * NB:(s + 1) * NB]                      # [NB, N, DI]
        x_in = np.ascontiguousarray(
            xc.reshape(NB, NCH, P, DI).transpose(2, 0, 1, 3)).astype(bf16)
        xT_in = np.ascontiguousarray(
            xc.reshape(NB, N, ICH, P).transpose(3, 0, 2, 1)).astype(bf16)
        xb = xc.sum(axis=1) / C                          # [NB, DI] fp32
        xbar_in = np.ascontiguousarray(
            xb.reshape(NB, ICH, P).transpose(2, 1, 0)).astype(bf16)
        in_maps.append(
            {"x": x_in, "xT": xT_in, "kk": kk, "kt": kt, "kkb": kkb,
             "xbar": xbar_in}
        )
    return in_maps


def kernel(x, kernel, _trace=False, _reps=0):
    from concourse.bass_utils import run_bass_kernel_spmd

    x = np.ascontiguousarray(np.asarray(x, dtype=np.float32))
    kernel = np.ascontiguousarray(np.asarray(kernel, dtype=np.float32))
    assert x.shape == (B, N, DI) and kernel.shape == (DI, C * D)

    key = ("nc", _reps)
    if key not in _cache:
        _cache[key] = _build_program(reps=_reps)
    nc = _cache[key]

    in_maps = _prep_inputs(x, kernel)
    res = run_bass_kernel_spmd(nc, in_maps, list(range(NCORES)), trace=_trace)
    _cache["last_result"] = res

    out = np.empty((B, C, D), dtype=np.float32)
    for s in range(NCORES):
        o = res.results[s]["out"]                        # [d, (c,b)]
        out[s * NB:(s + 1) * NB] = o.reshape(D, C, NB).transpose(2, 1, 0)
    return out



# revision 17
# speedup vs baseline: 1.5124x; 1.5124x over previous
"""Trainium2 Bass kernel for capsule dynamic routing (nn_Capsule).

Reference computation:
    hat = (x @ kernel).reshape(B, N, C, D).transpose(0, 2, 1, 3)   # [B,C,N,D]
    b = 0; 3 routing iterations of:
        w = softmax(b, axis=capsules)
        o = squash(einsum('bcn,bcnd->bcd', w, hat))
        b += einsum('bcd,bcnd->bcn', o, hat)

Key reformulation (hat is never materialized):
    o[c,d]  = sum_i xw[c,i] * K[i,(c,d)]      xw = w[c,:] @ x      (A-op)
    bupd[c,n] = sum_i x[n,i] * oK[c,i]        oK[c,i] = sum_d o[c,d]*K[i,(c,d)]
This reduces 34 GFLOP of hat-work to ~100 MFLOP of small matmuls whose cost
is streaming K through the PE as stationary operands (bf16, fp32 accumulate).

Sharding: data-parallel over batch B=16 across 8 cores (2 items/core, fused
into the same matmuls via a 2-wide moving operand). kernel K replicated.

Precision/speed split: the four routing K-passes (A0, oK0, A1, oK1) only
perturb softmax logits, so their stationary K operands are fp8 e3m4 (4-bit
mantissa, host-prescaled by S8 to fill the e3m4 range) — fp8 weight loads
run 2x faster through the PE (FWL loads 4 fp8 cols/cycle vs 2 bf16). The
final A2 pass feeds the output directly and keeps a bf16 K copy. The 1/S8^2
unscale folds into the squash-scale activations (Square scale=1/S8, Exp
bias=-2*ln(S8)), costing zero extra instructions.
"""

import math
import numpy as np
import ml_dtypes
from contextlib import ExitStack

NCORES = 8
B, N, DI = 16, 512, 256         # batch, input capsules, input dim
C, D = 64, 128                  # output capsules, capsule dim
NB = B // NCORES                # batch items per core
P = 128                         # SBUF partitions
NCH = N // P                    # 4 n-chunks
ICH = DI // P                   # 2 i-chunks
ROUTINGS = 3
EPS = 1e-7
S8 = 512.0                      # host pre-scale on K before e3m4 cast

_cache = {}


def _build_program(reps=0, no_dma=False):
    """reps=0: plain single-shot program (graded path).
    reps>0: wrap the whole body (input DMA + compute + output DMA) in a
    For_i loop for wall-clock-difference benchmarking.
    no_dma=True (probe): hoist the input DMAs out of the reps loop so the
    loop times pure compute."""
    import concourse.bass_isa as bass_isa
    import concourse.mybir as mybir
    import concourse.tile as tile
    from concourse import bacc

    F32 = mybir.dt.float32
    BF16 = mybir.dt.bfloat16
    FP8 = mybir.dt.float8e3
    AF = mybir.ActivationFunctionType
    AX = mybir.AxisListType

    class _OneActSetBacc(bacc.Bacc):
        """Every activation func used here (Square/Ln/Exp/Copy) lives in the
        'natural_log_exp_and_others' table set, but the default chooser picks
        per-func sets greedily and flip-flops (one ~1.3us LoadActFuncSet per
        switch, on the critical path). Mask the other sets so exactly one
        table load is emitted; indices are preserved so act_func_set_id still
        points at the real act_info.json entry."""

        def insert_act_table_loads(self):
            import bass_rust as _br
            from concourse.hw_specs import get_activation_tables

            has_activation = any(
                isinstance(i, mybir.InstActivation)
                for b in self.main_func.blocks
                for i in b.instructions
            )
            if not has_activation:
                return
            tables = [
                (name, funcs if name == "natural_log_exp_and_others" else set())
                for name, funcs in get_activation_tables(self.m.arch).items()
            ]
            _br.insert_act_table_loads(self, tables)

    nc = _OneActSetBacc("TRN2", target_bir_lowering=False, debug=False)

    x_d = nc.dram_tensor("x", [P, NB, NCH, DI], BF16, kind="ExternalInput").ap()
    xT_d = nc.dram_tensor("xT", [P, NB, ICH, N], BF16, kind="ExternalInput").ap()
    kk_d = nc.dram_tensor("kk", [P, ICH, C * D], FP8, kind="ExternalInput").ap()
    kt_d = nc.dram_tensor("kt", [P, C, DI], FP8, kind="ExternalInput").ap()
    kkb_d = nc.dram_tensor("kkb", [P, ICH, C * D], BF16, kind="ExternalInput").ap()
    xbar_d = nc.dram_tensor("xbar", [P, ICH, NB], BF16, kind="ExternalInput").ap()
    # output is [d, (c,b)]; the host does the final transpose to [b, c, d]
    out_d = nc.dram_tensor("out", [P, C * NB], F32, kind="ExternalOutput").ap()

    with tile.TileContext(nc) as tc, ExitStack() as ctx:
        big = ctx.enter_context(tc.tile_pool(name="big", bufs=1))
        wk = ctx.enter_context(tc.tile_pool(name="wk", bufs=2))
        psA = ctx.enter_context(tc.tile_pool(name="psA", bufs=2, space="PSUM"))
        psk = ctx.enter_context(tc.tile_pool(name="psk", bufs=2, space="PSUM"))
        pss = ctx.enter_context(tc.tile_pool(name="pss", bufs=3, space="PSUM"))

        kk = big.tile([P, ICH, C * D], FP8)
        kt = big.tile([P, C, DI], FP8)
        kkb = big.tile([P, ICH, C * D], BF16)
        xs = big.tile([P, NB, NCH, DI], BF16)
        xT = big.tile([P, NB, ICH, N], BF16)
        xbar = big.tile([P, ICH, NB], BF16)

        def emit_input_dmas():
            # Two hardware DMA rings (SP + ACT hwdge), ~190 GB/s each
            # sustained: one ring alone serializes ~9MB/rep into the whole
            # kernel span. Balanced split, each ring in first-use order:
            #   SP:  xbar, xs, kkb (4.5MB)   kkb feeds only the late A2 pass
            #   ACT: kk, kt, xT    (4.25MB)  kk feeds A0 at t=0, kt feeds oK0
            # kk/kt/kkb are chunked so capsule matmuls start while later
            # chunks are still in flight.
            KCH = 16
            kw = (C * D) // KCH
            nc.sync.dma_start(out=xbar, in_=xbar_d)
            nc.sync.dma_start(out=xs, in_=x_d)
            for j in range(KCH):
                nc.sync.dma_start(
                    out=kkb[:, :, j * kw:(j + 1) * kw],
                    in_=kkb_d[:, :, j * kw:(j + 1) * kw],
                )
            for j in range(KCH):
                nc.scalar.dma_start(
                    out=kk[:, :, j * kw:(j + 1) * kw],
                    in_=kk_d[:, :, j * kw:(j + 1) * kw],
                )
            for j in range(KCH):
                cs = C // KCH
                nc.scalar.dma_start(
                    out=kt[:, j * cs:(j + 1) * cs, :],
                    in_=kt_d[:, j * cs:(j + 1) * cs, :],
                )
            nc.scalar.dma_start(out=xT, in_=xT_d)

        bT = big.tile([P, NB, NCH, C], F32)     # routing logits, [n, b, nch, c]
        o_bf = big.tile([P, C, NB], BF16)       # current (unscaled) o, [d, (c,b)]
        epst = big.tile([P, 1], F32)
        nc.vector.memset(epst, EPS)
        zerot = big.tile([P, 1], F32)
        nc.vector.memset(zerot, 0.0)
        m2ls = big.tile([P, 1], F32)            # -2*ln(S8): unscales fp8 passes
        nc.vector.memset(m2ls, -2.0 * math.log(S8))
        # Dummy activation up front so the one LoadActFuncSet (~1.3us) runs
        # during the initial DMA wait instead of on the critical path.
        warm = big.tile([P, 1], F32)
        nc.scalar.activation(out=warm, in_=zerot[:], func=AF.Exp, bias=zerot[:])

        def a_op(rhs_at, stat):
            """o_pre[d,(c,b)] = sum_i rhs[i,(c,b)] * K[i,(c,d)] per capsule.
            stat: kk (fp8, routing passes) or kkb (bf16, final pass)."""
            po = psA.tile([P, C, NB], F32, tag="po")
            for c in range(C):
                for t in range(ICH):
                    nc.tensor.matmul(
                        po[:, c, :],
                        lhsT=stat[:, t, c * D:(c + 1) * D],
                        rhs=rhs_at(t, c),
                        start=(t == 0),
                        stop=(t == ICH - 1),
                    )
            return po

        def squash_scale(po, scaled):
            """scale[c,b] = sqrt(s+eps)/(0.5+s+eps), s = sum_d o_pre[d,(c,b)]^2,
            computed in free layout [*, (c,b)] (identical rows) so it can be
            applied with free-dim broadcasts downstream. The squash scale
            commutes past the (linear) oK matmul, so the PE proceeds straight
            from the A-op into the oK matmuls while this runs on ACT/Pool/DVE.
            Ln/Exp/Square share one activation-table set (no reloads);
            sqrt(t) = exp(0.5*ln(t)).
            scaled=True: po holds S8*o_pre (fp8 pass). Square's scale=1/S8
            recovers the true S, and Exp's bias=-2*ln(S8) folds the 1/S8^2 the
            downstream oks eviction needs (oks = pk * f/S8^2 with pk scaled by
            S8^2). scaled=False (bf16 final pass): returns the true f."""
            po2 = po[:].rearrange("p c b -> p (c b)")
            sq = wk.tile([P, C * NB], F32, tag="sq")
            nc.scalar.activation(out=sq, in_=po2, func=AF.Square,
                                 scale=(1.0 / S8) if scaled else 1.0)
            S = wk.tile([P, C * NB], F32, tag="S")
            nc.gpsimd.partition_all_reduce(S, sq, P, bass_isa.ReduceOp.add)
            num = wk.tile([P, C * NB], F32, tag="num")
            nc.scalar.activation(out=num, in_=S, func=AF.Ln, bias=epst[:])
            nc.scalar.activation(out=num, in_=num, func=AF.Exp, scale=0.5,
                                 bias=m2ls[:] if scaled else zerot[:])
            den = wk.tile([P, C * NB], F32, tag="den")
            nc.vector.tensor_scalar_add(den, S, 0.5 + EPS)
            nc.vector.reciprocal(den, den)
            scale = wk.tile([P, C * NB], F32, tag="scalef")
            nc.vector.tensor_mul(scale, num, den)
            return scale

        def b_op(it, scale):
            # oK_pre[i,(c,b)] per i-tile: contraction over d on partitions.
            # t-outer order: chunk-0 results complete at half-pass so the oks
            # evictions (DVE) overlap the chunk-1 matmuls.
            pk = psk.tile([P, ICH, C, NB], F32, tag="pk")
            for t in range(ICH):
                for c in range(C):
                    nc.tensor.matmul(
                        pk[:, t, c, :],
                        lhsT=kt[:, c, t * P:(t + 1) * P],
                        rhs=o_bf[:, c, :],
                        start=True,
                        stop=True,
                    )
            # apply the squash scale during the PSUM->SBUF eviction; split per
            # (b, i-tile) so each bupd matmul starts as soon as its slice lands
            oks = wk.tile([P, NB, ICH, C], BF16, tag="oks")
            sc3 = scale[:].rearrange("p (c b) -> p b c", b=NB)
            for t in range(ICH):
                for b in range(NB):
                    nc.vector.tensor_tensor(
                        oks[:, b, t, :], pk[:, t, :, b], sc3[:, b, :],
                        mybir.AluOpType.mult,
                    )
            # bupd[n,(c)] = sum_i x[n,i] oK[c,i]  (lhsT = xT tiles); all four
            # n-tiles of one batch item share a PSUM tile -> one eviction.
            pbs = []
            for b in range(NB):
                pb = pss.tile([P, NCH, C], F32, tag="pb")
                for nt in range(NCH):
                    for t in range(ICH):
                        nc.tensor.matmul(
                            pb[:, nt, :],
                            lhsT=xT[:, b, t, nt * P:(nt + 1) * P],
                            rhs=oks[:, b, t, :],
                            start=(t == 0),
                            stop=(t == ICH - 1),
                        )
                pbs.append(pb)
                if it == 0:
                    # exp can read straight from PSUM (ACT) in parallel with
                    # the DVE copy into bT, instead of serially after it
                    nc.vector.tensor_copy(out=bT[:, b], in_=pb)
                else:
                    nc.vector.tensor_add(out=bT[:, b], in0=bT[:, b], in1=pb)
            return pbs

        def softmax_xw(it, pbs):
            # softmax over capsules (innermost free axis of bT), split
            # per batch item so item 0's softmax (ACT/DVE) hides under item
            # 1's bupd matmuls, and the xw matmuls start as soon as their
            # item's weights are ready. Values are O(1) so the max-
            # subtraction is unnecessary.
            e = wk.tile([P, NB, NCH, C], F32, tag="e")
            es = wk.tile([P, NB, NCH], F32, tag="es")
            w = wk.tile([P, NB, NCH, C], BF16, tag="w")
            for b in range(NB):
                if it == 0:
                    nc.scalar.activation(out=e[:, b], in_=pbs[b][:],
                                         func=AF.Exp, bias=zerot[:])
                else:
                    nc.scalar.activation(out=e[:, b], in_=bT[:, b],
                                         func=AF.Exp, bias=zerot[:])
                nc.vector.reduce_sum(out=es[:, b], in_=e[:, b], axis=AX.X)
                nc.vector.reciprocal(es[:, b], es[:, b])
                nc.vector.tensor_tensor(
                    w[:, b], e[:, b],
                    es[:, b, :, None].to_broadcast((P, NCH, C)),
                    mybir.AluOpType.mult,
                )
            # xwT[i,(c,b)] = sum_n x[n,i] w[n,c]  (lhsT = x tiles); both
            # i-tiles of one batch item share a PSUM tile -> one eviction.
            xwT = wk.tile([P, ICH, C, NB], BF16, tag="xwT")
            for b in range(NB):
                px = pss.tile([P, ICH, C], F32, tag="pb")
                for t in range(ICH):
                    for ch in range(NCH):
                        nc.tensor.matmul(
                            px[:, t, :],
                            lhsT=xs[:, b, ch, t * P:(t + 1) * P],
                            rhs=w[:, b, ch, :],
                            start=(ch == 0),
                            stop=(ch == NCH - 1),
                        )
                nc.vector.tensor_copy(out=xwT[:, :, :, b], in_=px)
            return xwT

        def body():
            if not no_dma:
                emit_input_dmas()
            po = a_op(lambda t, c: xbar[:, t, :], kk)
            for it in range(ROUTINGS - 1):
                # chunked so the first oK matmuls start after ~1/4 of the copy
                for q in range(4):
                    cq = C // 4
                    nc.vector.tensor_copy(
                        out=o_bf[:, q * cq:(q + 1) * cq, :].rearrange(
                            "p c b -> p (c b)"),
                        in_=po[:, q * cq:(q + 1) * cq, :].rearrange(
                            "p c b -> p (c b)"),
                    )
                scale = squash_scale(po, scaled=True)
                pbs = b_op(it, scale)
                xwT = softmax_xw(it, pbs)
                last = it == ROUTINGS - 2
                po = a_op(lambda t, c, _x=xwT: _x[:, t, c, :],
                          kkb if last else kk)
            # final squash: o = o_pre * scale, emitted as [d, (c,b)]
            scale = squash_scale(po, scaled=False)
            oout = wk.tile([P, C * NB], F32, tag="oout")
            nc.vector.tensor_mul(oout, po[:].rearrange("p c b -> p (c b)"), scale)
            nc.sync.dma_start(out=out_d, in_=oout)

        if reps:
            if no_dma:
                emit_input_dmas()
            with tc.For_i(0, reps, 1, hint_engines=(mybir.EngineType.PE,)):
                body()
        else:
            body()

    nc.compile()
    return nc


def _prep_inputs(x, kernel):
    bf16 = ml_dtypes.bfloat16
    e3m4 = ml_dtypes.float8_e3m4
    # single consistent e3m4 quantization of S8*K, then rearranged into the
    # two stationary layouts so kk and kt hold identical values
    k8 = (kernel * np.float32(S8)).astype(e3m4)
    kk = np.ascontiguousarray(
        k8.reshape(ICH, P, C * D).transpose(1, 0, 2))
    kt = np.ascontiguousarray(
        k8.reshape(DI, C, D).transpose(2, 1, 0))
    kkb = np.ascontiguousarray(
        kernel.reshape(ICH, P, C * D).transpose(1, 0, 2)).astype(bf16)
    in_maps = []
    for s in range(NCORES):
        xc = x[s * NB:(s + 1) * NB]                      # [NB, N, DI]
        x_in = np.ascontiguousarray(
            xc.reshape(NB, NCH, P, DI).transpose(2, 0, 1, 3)).astype(bf16)
        xT_in = np.ascontiguousarray(
            xc.reshape(NB, N, ICH, P).transpose(3, 0, 2, 1)).astype(bf16)
        xb = xc.sum(axis=1) / C                          # [NB, DI] fp32
        xbar_in = np.ascontiguousarray(
            xb.reshape(NB, ICH, P).transpose(2, 1, 0)).astype(bf16)
        in_maps.append(
            {"x": x_in, "xT": xT_in, "kk": kk, "kt": kt, "kkb": kkb,
             "xbar": xbar_in}
        )
    return in_maps


def kernel(x, kernel, _trace=False, _reps=0, _no_dma=False):
    from concourse.bass_utils import run_bass_kernel_spmd

    x = np.ascontiguousarray(np.asarray(x, dtype=np.float32))
    kernel = np.ascontiguousarray(np.asarray(kernel, dtype=np.float32))
    assert x.shape == (B, N, DI) and kernel.shape == (DI, C * D)

    key = ("nc", _reps, _no_dma)
    if key not in _cache:
        _cache[key] = _build_program(reps=_reps, no_dma=_no_dma)
    nc = _cache[key]

    in_maps = _prep_inputs(x, kernel)
    res = run_bass_kernel_spmd(nc, in_maps, list(range(NCORES)), trace=_trace)
    _cache["last_result"] = res

    out = np.empty((B, C, D), dtype=np.float32)
    for s in range(NCORES):
        o = res.results[s]["out"]                        # [d, (c,b)]
        out[s * NB:(s + 1) * NB] = o.reshape(D, C, NB).transpose(2, 1, 0)
    return out



# revision 23
# speedup vs baseline: 1.6075x; 1.0629x over previous
"""Trainium2 Bass kernel for capsule dynamic routing (nn_Capsule).

Reference computation:
    hat = (x @ kernel).reshape(B, N, C, D).transpose(0, 2, 1, 3)   # [B,C,N,D]
    b = 0; 3 routing iterations of:
        w = softmax(b, axis=capsules)
        o = squash(einsum('bcn,bcnd->bcd', w, hat))
        b += einsum('bcd,bcnd->bcn', o, hat)

Key reformulation (hat is never materialized):
    o[c,d]  = sum_i xw[c,i] * K[i,(c,d)]      xw = w[c,:] @ x      (A-op)
    bupd[c,n] = sum_i x[n,i] * oK[c,i]        oK[c,i] = sum_d o[c,d]*K[i,(c,d)]
This reduces 34 GFLOP of hat-work to ~100 MFLOP of small matmuls whose cost
is streaming K through the PE as stationary operands (bf16, fp32 accumulate).

Sharding: data-parallel over batch B=16 across 8 cores (2 items/core, fused
into the same matmuls via a 2-wide moving operand). kernel K replicated.

Precision/speed split: the four routing K-passes (A0, oK0, A1, oK1) only
perturb softmax logits, so their stationary K operands are fp8 e3m4 (4-bit
mantissa, host-prescaled by S8 to fill the e3m4 range) — fp8 weight loads
run 2x faster through the PE (FWL loads 4 fp8 cols/cycle vs 2 bf16). The
final A2 pass feeds the output directly and keeps a bf16 K copy. The 1/S8^2
unscale folds into the squash-scale activations (Square scale=1/S8, Exp
bias=-2*ln(S8)), costing zero extra instructions.
"""

import math
import numpy as np
import ml_dtypes
from contextlib import ExitStack

NCORES = 8
B, N, DI = 16, 512, 256         # batch, input capsules, input dim
C, D = 64, 128                  # output capsules, capsule dim
NB = B // NCORES                # batch items per core
P = 128                         # SBUF partitions
NCH = N // P                    # 4 n-chunks
ICH = DI // P                   # 2 i-chunks
ROUTINGS = 3
EPS = 1e-7
S8 = 512.0                      # host pre-scale on K before e3m4 cast

_cache = {}


def _build_program(reps=0, no_dma=False):
    """reps=0: plain single-shot program (graded path).
    reps>0: wrap the whole body (input DMA + compute + output DMA) in a
    For_i loop for wall-clock-difference benchmarking.
    no_dma=True (probe): hoist the input DMAs out of the reps loop so the
    loop times pure compute."""
    import concourse.bass_isa as bass_isa
    import concourse.mybir as mybir
    import concourse.tile as tile
    from concourse import bacc

    F32 = mybir.dt.float32
    BF16 = mybir.dt.bfloat16
    FP8 = mybir.dt.float8e3
    AF = mybir.ActivationFunctionType
    AX = mybir.AxisListType

    class _OneActSetBacc(bacc.Bacc):
        """Every activation func used here (Square/Ln/Exp/Copy) lives in the
        'natural_log_exp_and_others' table set, but the default chooser picks
        per-func sets greedily and flip-flops (one ~1.3us LoadActFuncSet per
        switch, on the critical path). Mask the other sets so exactly one
        table load is emitted; indices are preserved so act_func_set_id still
        points at the real act_info.json entry."""

        def insert_act_table_loads(self):
            import bass_rust as _br
            from concourse.hw_specs import get_activation_tables

            has_activation = any(
                isinstance(i, mybir.InstActivation)
                for b in self.main_func.blocks
                for i in b.instructions
            )
            if not has_activation:
                return
            tables = [
                (name, funcs if name == "natural_log_exp_and_others" else set())
                for name, funcs in get_activation_tables(self.m.arch).items()
            ]
            _br.insert_act_table_loads(self, tables)

    nc = _OneActSetBacc("TRN2", target_bir_lowering=False, debug=False)

    x_d = nc.dram_tensor("x", [P, NB, NCH, DI], BF16, kind="ExternalInput").ap()
    xT_d = nc.dram_tensor("xT", [P, NB, ICH, N], FP8, kind="ExternalInput").ap()
    kk_d = nc.dram_tensor("kk", [P, ICH, C * D], FP8, kind="ExternalInput").ap()
    kt_d = nc.dram_tensor("kt", [P, C, DI], FP8, kind="ExternalInput").ap()
    kkb_d = nc.dram_tensor("kkb", [P, ICH, C * D], BF16, kind="ExternalInput").ap()
    xbar_d = nc.dram_tensor("xbar", [P, ICH, NB], BF16, kind="ExternalInput").ap()
    # output is [d, (c,b)]; the host does the final transpose to [b, c, d]
    out_d = nc.dram_tensor("out", [P, C * NB], F32, kind="ExternalOutput").ap()

    with tile.TileContext(nc) as tc, ExitStack() as ctx:
        big = ctx.enter_context(tc.tile_pool(name="big", bufs=1))
        wk = ctx.enter_context(tc.tile_pool(name="wk", bufs=2))
        # psA=3: the three A-pass outputs (A0/A1/A2) get distinct PSUM
        # buffers, so rep i+1's A0 doesn't WAR-wait on rep i's final squash
        # still reading A2's output. pss=4: pb(b0),pb(b1),px(b0),px(b1)
        # rotate without reuse within an iteration.
        psA = ctx.enter_context(tc.tile_pool(name="psA", bufs=3, space="PSUM"))
        psk = ctx.enter_context(tc.tile_pool(name="psk", bufs=1, space="PSUM"))
        pss = ctx.enter_context(tc.tile_pool(name="pss", bufs=4, space="PSUM"))

        kk = big.tile([P, ICH, C * D], FP8)
        kt = big.tile([P, C, DI], FP8)
        kkb = big.tile([P, ICH, C * D], BF16)
        xs = big.tile([P, NB, NCH, DI], BF16)
        xT = big.tile([P, NB, ICH, N], FP8)
        xbar = big.tile([P, ICH, NB], BF16)

        def emit_input_dmas():
            # One DMA ring sustains only ~217 GB/s, so 9MB/rep serializes
            # into the whole kernel span. Split across two rings: the SP
            # hwdge ring carries everything needed early (first-use order);
            # the gpsimd SWDGE ring carries kkb (4MB), which is only read by
            # the late A2 pass — and the gpsimd queue runs just three small
            # reduces per rep, so the enqueues don't block compute. (An ACT-
            # ring split was tried and head-of-line-blocked the activations:
            # 96us vs 61us.)
            KCH = 16
            kw = (C * D) // KCH
            nc.sync.dma_start(out=xbar, in_=xbar_d)
            for j in range(KCH):
                nc.sync.dma_start(
                    out=kk[:, :, j * kw:(j + 1) * kw],
                    in_=kk_d[:, :, j * kw:(j + 1) * kw],
                )
            for j in range(KCH):
                cs = C // KCH
                nc.sync.dma_start(
                    out=kt[:, j * cs:(j + 1) * cs, :],
                    in_=kt_d[:, j * cs:(j + 1) * cs, :],
                )
            nc.sync.dma_start(out=xT, in_=xT_d)
            nc.sync.dma_start(out=xs, in_=x_d)
            for j in range(KCH):
                nc.gpsimd.dma_start(
                    out=kkb[:, :, j * kw:(j + 1) * kw],
                    in_=kkb_d[:, :, j * kw:(j + 1) * kw],
                )

        bT = big.tile([P, NB, NCH, C], F32)     # routing logits, [n, b, nch, c]
        o_bf = big.tile([P, C, NB], BF16)       # current (unscaled) o, [d, (c,b)]
        epst = big.tile([P, 1], F32)
        nc.vector.memset(epst, EPS)
        zerot = big.tile([P, 1], F32)
        nc.vector.memset(zerot, 0.0)
        m2ls = big.tile([P, 1], F32)            # -2*ln(S8): unscales fp8 passes
        nc.vector.memset(m2ls, -2.0 * math.log(S8))
        # Dummy activation up front so the one LoadActFuncSet (~1.3us) runs
        # during the initial DMA wait instead of on the critical path.
        warm = big.tile([P, 1], F32)
        nc.scalar.activation(out=warm, in_=zerot[:], func=AF.Exp, bias=zerot[:])

        def a_op(rhs_at, stat):
            """o_pre[d,(c,b)] = sum_i rhs[i,(c,b)] * K[i,(c,d)] per capsule.
            stat: kk (fp8, routing passes) or kkb (bf16, final pass)."""
            po = psA.tile([P, C, NB], F32, tag="po")
            for c in range(C):
                for t in range(ICH):
                    nc.tensor.matmul(
                        po[:, c, :],
                        lhsT=stat[:, t, c * D:(c + 1) * D],
                        rhs=rhs_at(t, c),
                        start=(t == 0),
                        stop=(t == ICH - 1),
                    )
            return po

        def squash_scale(po, scaled):
            """scale[c,b] = sqrt(s+eps)/(0.5+s+eps), s = sum_d o_pre[d,(c,b)]^2,
            computed in free layout [*, (c,b)] (identical rows) so it can be
            applied with free-dim broadcasts downstream. The squash scale
            commutes past the (linear) oK matmul, so the PE proceeds straight
            from the A-op into the oK matmuls while this runs on ACT/Pool/DVE.
            Ln/Exp/Square share one activation-table set (no reloads);
            sqrt(t) = exp(0.5*ln(t)).
            scaled=True: po holds S8*o_pre (fp8 pass). Square's scale=1/S8
            recovers the true S, and Exp's bias=-2*ln(S8) folds the 1/S8^2 the
            downstream oks eviction needs (oks = pk * f/S8^2 with pk scaled by
            S8^2). scaled=False (bf16 final pass): returns the true f."""
            po2 = po[:].rearrange("p c b -> p (c b)")
            sq = wk.tile([P, C * NB], F32, tag="sq")
            nc.scalar.activation(out=sq, in_=po2, func=AF.Square,
                                 scale=(1.0 / S8) if scaled else 1.0)
            S = wk.tile([P, C * NB], F32, tag="S")
            nc.gpsimd.partition_all_reduce(S, sq, P, bass_isa.ReduceOp.add)
            num = wk.tile([P, C * NB], F32, tag="num")
            nc.scalar.activation(out=num, in_=S, func=AF.Ln, bias=epst[:])
            nc.scalar.activation(out=num, in_=num, func=AF.Exp, scale=0.5,
                                 bias=m2ls[:] if scaled else zerot[:])
            den = wk.tile([P, C * NB], F32, tag="den")
            nc.vector.tensor_scalar_add(den, S, 0.5 + EPS)
            nc.vector.reciprocal(den, den)
            scale = wk.tile([P, C * NB], F32, tag="scalef")
            nc.vector.tensor_mul(scale, num, den)
            return scale

        def b_op(it, scale):
            # oK_pre[i,(c,b)] per i-tile: contraction over d on partitions.
            # t-outer order: chunk-0 results complete at half-pass so the oks
            # evictions (DVE) overlap the chunk-1 matmuls.
            pk = psk.tile([P, ICH, C, NB], F32, tag="pk")
            for t in range(ICH):
                for c in range(C):
                    nc.tensor.matmul(
                        pk[:, t, c, :],
                        lhsT=kt[:, c, t * P:(t + 1) * P],
                        rhs=o_bf[:, c, :],
                        start=True,
                        stop=True,
                    )
            # apply the squash scale during the PSUM->SBUF eviction; split per
            # (b, i-tile) so each bupd matmul starts as soon as its slice lands
            oks = wk.tile([P, NB, ICH, C], BF16, tag="oks")
            sc3 = scale[:].rearrange("p (c b) -> p b c", b=NB)
            for t in range(ICH):
                for b in range(NB):
                    nc.vector.tensor_tensor(
                        oks[:, b, t, :], pk[:, t, :, b], sc3[:, b, :],
                        mybir.AluOpType.mult,
                    )
            # bupd[n,(c)] = sum_i x[n,i] oK[c,i]  (lhsT = xT tiles); all four
            # n-tiles of one batch item share a PSUM tile -> one eviction.
            pbs = []
            for b in range(NB):
                pb = pss.tile([P, NCH, C], F32, tag="pb")
                for nt in range(NCH):
                    for t in range(ICH):
                        nc.tensor.matmul(
                            pb[:, nt, :],
                            lhsT=xT[:, b, t, nt * P:(nt + 1) * P],
                            rhs=oks[:, b, t, :],
                            start=(t == 0),
                            stop=(t == ICH - 1),
                        )
                pbs.append(pb)
                if it == 0:
                    # exp can read straight from PSUM (ACT) in parallel with
                    # the DVE copy into bT, instead of serially after it
                    nc.vector.tensor_copy(out=bT[:, b], in_=pb)
                else:
                    nc.vector.tensor_add(out=bT[:, b], in0=bT[:, b], in1=pb)
            return pbs

        def softmax_xw(it, pbs):
            # softmax over capsules (innermost free axis of bT), split
            # per batch item so item 0's softmax (ACT/DVE) hides under item
            # 1's bupd matmuls, and the xw matmuls start as soon as their
            # item's weights are ready. Values are O(1) so the max-
            # subtraction is unnecessary.
            e = wk.tile([P, NB, NCH, C], F32, tag="e")
            es = wk.tile([P, NB, NCH], F32, tag="es")
            w = wk.tile([P, NB, NCH, C], BF16, tag="w")
            for b in range(NB):
                if it == 0:
                    nc.scalar.activation(out=e[:, b], in_=pbs[b][:],
                                         func=AF.Exp, bias=zerot[:])
                else:
                    nc.scalar.activation(out=e[:, b], in_=bT[:, b],
                                         func=AF.Exp, bias=zerot[:])
                nc.vector.reduce_sum(out=es[:, b], in_=e[:, b], axis=AX.X)
                nc.vector.reciprocal(es[:, b], es[:, b])
                nc.vector.tensor_tensor(
                    w[:, b], e[:, b],
                    es[:, b, :, None].to_broadcast((P, NCH, C)),
                    mybir.AluOpType.mult,
                )
            # xwT[i,(c,b)] = sum_n x[n,i] w[n,c]  (lhsT = x tiles); both
            # i-tiles of one batch item share a PSUM tile -> one eviction.
            xwT = wk.tile([P, ICH, C, NB], BF16, tag="xwT")
            for b in range(NB):
                px = pss.tile([P, ICH, C], F32, tag="pb")
                for t in range(ICH):
                    for ch in range(NCH):
                        nc.tensor.matmul(
                            px[:, t, :],
                            lhsT=xs[:, b, ch, t * P:(t + 1) * P],
                            rhs=w[:, b, ch, :],
                            start=(ch == 0),
                            stop=(ch == NCH - 1),
                        )
                nc.vector.tensor_copy(out=xwT[:, :, :, b], in_=px)
            return xwT

        def body():
            if not no_dma:
                emit_input_dmas()
            po = a_op(lambda t, c: xbar[:, t, :], kk)
            for it in range(ROUTINGS - 1):
                # chunked so the first oK matmuls start after ~1/4 of the copy
                for q in range(4):
                    cq = C // 4
                    nc.vector.tensor_copy(
                        out=o_bf[:, q * cq:(q + 1) * cq, :].rearrange(
                            "p c b -> p (c b)"),
                        in_=po[:, q * cq:(q + 1) * cq, :].rearrange(
                            "p c b -> p (c b)"),
                    )
                scale = squash_scale(po, scaled=True)
                pbs = b_op(it, scale)
                xwT = softmax_xw(it, pbs)
                last = it == ROUTINGS - 2
                po = a_op(lambda t, c, _x=xwT: _x[:, t, c, :],
                          kkb if last else kk)
            # final squash: o = o_pre * scale, emitted as [d, (c,b)]
            scale = squash_scale(po, scaled=False)
            oout = wk.tile([P, C * NB], F32, tag="oout")
            nc.vector.tensor_mul(oout, po[:].rearrange("p c b -> p (c b)"), scale)
            nc.sync.dma_start(out=out_d, in_=oout)

        if reps:
            if no_dma:
                emit_input_dmas()
            with tc.For_i(0, reps, 1, hint_engines=(mybir.EngineType.PE,)):
                body()
        else:
            body()

    nc.compile()
    return nc


def _prep_inputs(x, kernel):
    bf16 = ml_dtypes.bfloat16
    e3m4 = ml_dtypes.float8_e3m4
    # single consistent e3m4 quantization of S8*K, then rearranged into the
    # two stationary layouts so kk and kt hold identical values
    k8 = (kernel * np.float32(S8)).astype(e3m4)
    kk = np.ascontiguousarray(
        k8.reshape(ICH, P, C * D).transpose(1, 0, 2))
    kt = np.ascontiguousarray(
        k8.reshape(DI, C, D).transpose(2, 1, 0))
    kkb = np.ascontiguousarray(
        kernel.reshape(ICH, P, C * D).transpose(1, 0, 2)).astype(bf16)
    in_maps = []
    for s in range(NCORES):
        xc = x[s * NB:(s + 1) * NB]                      # [NB, N, DI]
        x_in = np.ascontiguousarray(
            xc.reshape(NB, NCH, P, DI).transpose(2, 0, 1, 3)).astype(bf16)
        xT_in = np.ascontiguousarray(
            xc.reshape(NB, N, ICH, P).transpose(3, 0, 2, 1)).astype(e3m4)
        xb = xc.sum(axis=1) / C                          # [NB, DI] fp32
        xbar_in = np.ascontiguousarray(
            xb.reshape(NB, ICH, P).transpose(2, 1, 0)).astype(bf16)
        in_maps.append(
            {"x": x_in, "xT": xT_in, "kk": kk, "kt": kt, "kkb": kkb,
             "xbar": xbar_in}
        )
    return in_maps


def kernel(x, kernel, _trace=False, _reps=0, _no_dma=False):
    from concourse.bass_utils import run_bass_kernel_spmd

    x = np.ascontiguousarray(np.asarray(x, dtype=np.float32))
    kernel = np.ascontiguousarray(np.asarray(kernel, dtype=np.float32))
    assert x.shape == (B, N, DI) and kernel.shape == (DI, C * D)

    key = ("nc", _reps, _no_dma)
    if key not in _cache:
        _cache[key] = _build_program(reps=_reps, no_dma=_no_dma)
    nc = _cache[key]

    in_maps = _prep_inputs(x, kernel)
    res = run_bass_kernel_spmd(nc, in_maps, list(range(NCORES)), trace=_trace)
    _cache["last_result"] = res

    out = np.empty((B, C, D), dtype=np.float32)
    for s in range(NCORES):
        o = res.results[s]["out"]                        # [d, (c,b)]
        out[s * NB:(s + 1) * NB] = o.reshape(D, C, NB).transpose(2, 1, 0)
    return out



# revision 25
# speedup vs baseline: 1.9493x; 1.2127x over previous
"""Trainium2 Bass kernel for capsule dynamic routing (nn_Capsule).

Reference computation:
    hat = (x @ kernel).reshape(B, N, C, D).transpose(0, 2, 1, 3)   # [B,C,N,D]
    b = 0; 3 routing iterations of:
        w = softmax(b, axis=capsules)
        o = squash(einsum('bcn,bcnd->bcd', w, hat))
        b += einsum('bcd,bcnd->bcn', o, hat)

Key reformulation (hat is never materialized):
    o[c,d]  = sum_i xw[c,i] * K[i,(c,d)]      xw = w[c,:] @ x      (A-op)
    bupd[c,n] = sum_i x[n,i] * oK[c,i]        oK[c,i] = sum_d o[c,d]*K[i,(c,d)]
This reduces 34 GFLOP of hat-work to ~100 MFLOP of small matmuls whose cost
is streaming K through the PE as stationary operands (bf16, fp32 accumulate).

Sharding: data-parallel over batch B=16 across 8 cores (2 items/core, fused
into the same matmuls via a 2-wide moving operand). kernel K replicated.

Precision/speed split: the four routing K-passes (A0, oK0, A1, oK1) only
perturb softmax logits, so their stationary K operands are fp8 e3m4 (4-bit
mantissa, host-prescaled by S8 to fill the e3m4 range) — fp8 weight loads
run 2x faster through the PE (FWL loads 4 fp8 cols/cycle vs 2 bf16). The
final A2 pass feeds the output directly and keeps a bf16 K copy. The 1/S8^2
unscale folds into the squash-scale activations (Square scale=1/S8, Exp
bias=-2*ln(S8)), costing zero extra instructions.
"""

import math
import numpy as np
import ml_dtypes
from contextlib import ExitStack

NCORES = 8
B, N, DI = 16, 512, 256         # batch, input capsules, input dim
C, D = 64, 128                  # output capsules, capsule dim
NB = B // NCORES                # batch items per core
P = 128                         # SBUF partitions
NCH = N // P                    # 4 n-chunks
ICH = DI // P                   # 2 i-chunks
ROUTINGS = 3
EPS = 1e-7
S8 = 512.0                      # host pre-scale on K before e3m4 cast

_cache = {}


def _build_program(reps=0, no_dma=False):
    """reps=0: plain single-shot program (graded path).
    reps>0: wrap the whole body (input DMA + compute + output DMA) in a
    For_i loop for wall-clock-difference benchmarking.
    no_dma=True (probe): hoist the input DMAs out of the reps loop so the
    loop times pure compute."""
    import concourse.bass_isa as bass_isa
    import concourse.mybir as mybir
    import concourse.tile as tile
    from concourse import bacc

    F32 = mybir.dt.float32
    BF16 = mybir.dt.bfloat16
    FP8 = mybir.dt.float8e3
    AF = mybir.ActivationFunctionType
    AX = mybir.AxisListType

    class _OneActSetBacc(bacc.Bacc):
        """Every activation func used here (Square/Ln/Exp/Copy) lives in the
        'natural_log_exp_and_others' table set, but the default chooser picks
        per-func sets greedily and flip-flops (one ~1.3us LoadActFuncSet per
        switch, on the critical path). Mask the other sets so exactly one
        table load is emitted; indices are preserved so act_func_set_id still
        points at the real act_info.json entry."""

        def insert_act_table_loads(self):
            import bass_rust as _br
            from concourse.hw_specs import get_activation_tables

            has_activation = any(
                isinstance(i, mybir.InstActivation)
                for b in self.main_func.blocks
                for i in b.instructions
            )
            if not has_activation:
                return
            tables = [
                (name, funcs if name == "natural_log_exp_and_others" else set())
                for name, funcs in get_activation_tables(self.m.arch).items()
            ]
            _br.insert_act_table_loads(self, tables)

    nc = _OneActSetBacc("TRN2", target_bir_lowering=False, debug=False)

    x_d = nc.dram_tensor("x", [P, NB, NCH, DI], BF16, kind="ExternalInput").ap()
    xT_d = nc.dram_tensor("xT", [P, NB, ICH, N], FP8, kind="ExternalInput").ap()
    kk_d = nc.dram_tensor("kk", [P, ICH, C * D], FP8, kind="ExternalInput").ap()
    kt_d = nc.dram_tensor("kt", [P, C, DI], FP8, kind="ExternalInput").ap()
    kkb_d = nc.dram_tensor("kkb", [P, ICH, C * D], BF16, kind="ExternalInput").ap()
    xbar_d = nc.dram_tensor("xbar", [P, ICH, NB], BF16, kind="ExternalInput").ap()
    # output is [d, (c,b)]; the host does the final transpose to [b, c, d]
    out_d = nc.dram_tensor("out", [P, C * NB], F32, kind="ExternalOutput").ap()

    with tile.TileContext(nc) as tc, ExitStack() as ctx:
        big = ctx.enter_context(tc.tile_pool(name="big", bufs=1))
        wk = ctx.enter_context(tc.tile_pool(name="wk", bufs=2))
        # psA=3: the three A-pass outputs (A0/A1/A2) get distinct PSUM
        # buffers, so rep i+1's A0 doesn't WAR-wait on rep i's final squash
        # still reading A2's output. pss=4: pb(b0),pb(b1),px(b0),px(b1)
        # rotate without reuse within an iteration.
        psA = ctx.enter_context(tc.tile_pool(name="psA", bufs=3, space="PSUM"))
        psk = ctx.enter_context(tc.tile_pool(name="psk", bufs=1, space="PSUM"))
        pss = ctx.enter_context(tc.tile_pool(name="pss", bufs=4, space="PSUM"))

        kk = big.tile([P, ICH, C * D], FP8)
        kt = big.tile([P, C, DI], FP8)
        kkb = big.tile([P, ICH, C * D], BF16)
        xs = big.tile([P, NB, NCH, DI], BF16)
        xT = big.tile([P, NB, ICH, N], FP8)
        xbar = big.tile([P, ICH, NB], BF16)

        def emit_input_dmas():
            # One DMA ring sustains only ~217 GB/s, so 9MB/rep serializes
            # into the whole kernel span; and each ring is FIFO, so a
            # transfer whose WAR hazard clears late blocks everything queued
            # behind it. Split across the two hwdge rings by WAR-clear time:
            #   SP:  kk, kt (4MB) — their rep-(i-1) readers (A1, oK1) finish
            #        at ~65-75% of the rep, so these are the late-clearing
            #        transfers, alone on their ring in first-use order.
            #   ACT: xbar, xT, xs, kkb, out (4.8MB) — xbar's WAR clears at
            #        ~12% (A0 done), so this ring streams from early in the
            #        rep; kkb feeds only the late A2 pass; out last.
            #   (gpsimd SWDGE was tried for kkb: software DGE is far too
            #   slow, 78us. ACT-ring carrying kk/kt was tried: their late
            #   WARs head-of-line-blocked the activations, 96us.)
            KCH = 16
            kw = (C * D) // KCH
            for j in range(KCH):
                nc.sync.dma_start(
                    out=kk[:, :, j * kw:(j + 1) * kw],
                    in_=kk_d[:, :, j * kw:(j + 1) * kw],
                )
            for j in range(KCH):
                cs = C // KCH
                nc.sync.dma_start(
                    out=kt[:, j * cs:(j + 1) * cs, :],
                    in_=kt_d[:, j * cs:(j + 1) * cs, :],
                )
            nc.scalar.dma_start(out=xbar, in_=xbar_d)
            nc.scalar.dma_start(out=xT, in_=xT_d)
            nc.scalar.dma_start(out=xs, in_=x_d)
            for j in range(KCH):
                nc.scalar.dma_start(
                    out=kkb[:, :, j * kw:(j + 1) * kw],
                    in_=kkb_d[:, :, j * kw:(j + 1) * kw],
                )

        bT = big.tile([P, NB, NCH, C], F32)     # routing logits, [n, b, nch, c]
        o_bf = big.tile([P, C, NB], BF16)       # current (unscaled) o, [d, (c,b)]
        epst = big.tile([P, 1], F32)
        nc.vector.memset(epst, EPS)
        zerot = big.tile([P, 1], F32)
        nc.vector.memset(zerot, 0.0)
        m2ls = big.tile([P, 1], F32)            # -2*ln(S8): unscales fp8 passes
        nc.vector.memset(m2ls, -2.0 * math.log(S8))
        # Dummy activation up front so the one LoadActFuncSet (~1.3us) runs
        # during the initial DMA wait instead of on the critical path.
        warm = big.tile([P, 1], F32)
        nc.scalar.activation(out=warm, in_=zerot[:], func=AF.Exp, bias=zerot[:])

        def a_op(rhs_at, stat):
            """o_pre[d,(c,b)] = sum_i rhs[i,(c,b)] * K[i,(c,d)] per capsule.
            stat: kk (fp8, routing passes) or kkb (bf16, final pass)."""
            po = psA.tile([P, C, NB], F32, tag="po")
            for c in range(C):
                for t in range(ICH):
                    nc.tensor.matmul(
                        po[:, c, :],
                        lhsT=stat[:, t, c * D:(c + 1) * D],
                        rhs=rhs_at(t, c),
                        start=(t == 0),
                        stop=(t == ICH - 1),
                    )
            return po

        def squash_scale(po, scaled):
            """scale[c,b] = sqrt(s+eps)/(0.5+s+eps), s = sum_d o_pre[d,(c,b)]^2,
            computed in free layout [*, (c,b)] (identical rows) so it can be
            applied with free-dim broadcasts downstream. The squash scale
            commutes past the (linear) oK matmul, so the PE proceeds straight
            from the A-op into the oK matmuls while this runs on ACT/Pool/DVE.
            Ln/Exp/Square share one activation-table set (no reloads);
            sqrt(t) = exp(0.5*ln(t)).
            scaled=True: po holds S8*o_pre (fp8 pass). Square's scale=1/S8
            recovers the true S, and Exp's bias=-2*ln(S8) folds the 1/S8^2 the
            downstream oks eviction needs (oks = pk * f/S8^2 with pk scaled by
            S8^2). scaled=False (bf16 final pass): returns the true f."""
            po2 = po[:].rearrange("p c b -> p (c b)")
            sq = wk.tile([P, C * NB], F32, tag="sq")
            nc.scalar.activation(out=sq, in_=po2, func=AF.Square,
                                 scale=(1.0 / S8) if scaled else 1.0)
            S = wk.tile([P, C * NB], F32, tag="S")
            nc.gpsimd.partition_all_reduce(S, sq, P, bass_isa.ReduceOp.add)
            num = wk.tile([P, C * NB], F32, tag="num")
            nc.scalar.activation(out=num, in_=S, func=AF.Ln, bias=epst[:])
            nc.scalar.activation(out=num, in_=num, func=AF.Exp, scale=0.5,
                                 bias=m2ls[:] if scaled else zerot[:])
            den = wk.tile([P, C * NB], F32, tag="den")
            nc.vector.tensor_scalar_add(den, S, 0.5 + EPS)
            nc.vector.reciprocal(den, den)
            scale = wk.tile([P, C * NB], F32, tag="scalef")
            nc.vector.tensor_mul(scale, num, den)
            return scale

        def b_op(it, scale):
            # oK_pre[i,(c,b)] per i-tile: contraction over d on partitions.
            # t-outer order: chunk-0 results complete at half-pass so the oks
            # evictions (DVE) overlap the chunk-1 matmuls.
            pk = psk.tile([P, ICH, C, NB], F32, tag="pk")
            for t in range(ICH):
                for c in range(C):
                    nc.tensor.matmul(
                        pk[:, t, c, :],
                        lhsT=kt[:, c, t * P:(t + 1) * P],
                        rhs=o_bf[:, c, :],
                        start=True,
                        stop=True,
                    )
            # apply the squash scale during the PSUM->SBUF eviction; split per
            # (b, i-tile) so each bupd matmul starts as soon as its slice lands
            oks = wk.tile([P, NB, ICH, C], BF16, tag="oks")
            sc3 = scale[:].rearrange("p (c b) -> p b c", b=NB)
            for t in range(ICH):
                for b in range(NB):
                    nc.vector.tensor_tensor(
                        oks[:, b, t, :], pk[:, t, :, b], sc3[:, b, :],
                        mybir.AluOpType.mult,
                    )
            # bupd[n,(c)] = sum_i x[n,i] oK[c,i]  (lhsT = xT tiles); all four
            # n-tiles of one batch item share a PSUM tile -> one eviction.
            pbs = []
            for b in range(NB):
                pb = pss.tile([P, NCH, C], F32, tag="pb")
                for nt in range(NCH):
                    for t in range(ICH):
                        nc.tensor.matmul(
                            pb[:, nt, :],
                            lhsT=xT[:, b, t, nt * P:(nt + 1) * P],
                            rhs=oks[:, b, t, :],
                            start=(t == 0),
                            stop=(t == ICH - 1),
                        )
                pbs.append(pb)
                if it == 0:
                    # exp can read straight from PSUM (ACT) in parallel with
                    # the DVE copy into bT, instead of serially after it
                    nc.vector.tensor_copy(out=bT[:, b], in_=pb)
                else:
                    nc.vector.tensor_add(out=bT[:, b], in0=bT[:, b], in1=pb)
            return pbs

        def softmax_xw(it, pbs):
            # softmax over capsules (innermost free axis of bT), split
            # per batch item so item 0's softmax (ACT/DVE) hides under item
            # 1's bupd matmuls, and the xw matmuls start as soon as their
            # item's weights are ready. Values are O(1) so the max-
            # subtraction is unnecessary.
            e = wk.tile([P, NB, NCH, C], F32, tag="e")
            es = wk.tile([P, NB, NCH], F32, tag="es")
            w = wk.tile([P, NB, NCH, C], BF16, tag="w")
            for b in range(NB):
                if it == 0:
                    nc.scalar.activation(out=e[:, b], in_=pbs[b][:],
                                         func=AF.Exp, bias=zerot[:])
                else:
                    nc.scalar.activation(out=e[:, b], in_=bT[:, b],
                                         func=AF.Exp, bias=zerot[:])
                nc.vector.reduce_sum(out=es[:, b], in_=e[:, b], axis=AX.X)
                nc.vector.reciprocal(es[:, b], es[:, b])
                nc.vector.tensor_tensor(
                    w[:, b], e[:, b],
                    es[:, b, :, None].to_broadcast((P, NCH, C)),
                    mybir.AluOpType.mult,
                )
            # xwT[i,(c,b)] = sum_n x[n,i] w[n,c]  (lhsT = x tiles); both
            # i-tiles of one batch item share a PSUM tile -> one eviction.
            xwT = wk.tile([P, ICH, C, NB], BF16, tag="xwT")
            for b in range(NB):
                px = pss.tile([P, ICH, C], F32, tag="pb")
                for t in range(ICH):
                    for ch in range(NCH):
                        nc.tensor.matmul(
                            px[:, t, :],
                            lhsT=xs[:, b, ch, t * P:(t + 1) * P],
                            rhs=w[:, b, ch, :],
                            start=(ch == 0),
                            stop=(ch == NCH - 1),
                        )
                nc.vector.tensor_copy(out=xwT[:, :, :, b], in_=px)
            return xwT

        def body():
            if not no_dma:
                emit_input_dmas()
            po = a_op(lambda t, c: xbar[:, t, :], kk)
            for it in range(ROUTINGS - 1):
                # chunked so the first oK matmuls start after ~1/4 of the copy
                for q in range(4):
                    cq = C // 4
                    nc.vector.tensor_copy(
                        out=o_bf[:, q * cq:(q + 1) * cq, :].rearrange(
                            "p c b -> p (c b)"),
                        in_=po[:, q * cq:(q + 1) * cq, :].rearrange(
                            "p c b -> p (c b)"),
                    )
                scale = squash_scale(po, scaled=True)
                pbs = b_op(it, scale)
                xwT = softmax_xw(it, pbs)
                last = it == ROUTINGS - 2
                po = a_op(lambda t, c, _x=xwT: _x[:, t, c, :],
                          kkb if last else kk)
            # final squash: o = o_pre * scale, emitted as [d, (c,b)]
            scale = squash_scale(po, scaled=False)
            oout = wk.tile([P, C * NB], F32, tag="oout")
            nc.vector.tensor_mul(oout, po[:].rearrange("p c b -> p (c b)"), scale)
            # out rides the ACT ring: on SP it would head-of-line-block the
            # next rep's kk/kt transfers behind the final-squash dependency
            nc.scalar.dma_start(out=out_d, in_=oout)

        if reps:
            if no_dma:
                emit_input_dmas()
            with tc.For_i(0, reps, 1, hint_engines=(mybir.EngineType.PE,)):
                body()
        else:
            body()

    nc.compile()
    return nc


def _prep_inputs(x, kernel):
    bf16 = ml_dtypes.bfloat16
    e3m4 = ml_dtypes.float8_e3m4
    # single consistent e3m4 quantization of S8*K, then rearranged into the
    # two stationary layouts so kk and kt hold identical values
    k8 = (kernel * np.float32(S8)).astype(e3m4)
    kk = np.ascontiguousarray(
        k8.reshape(ICH, P, C * D).transpose(1, 0, 2))
    kt = np.ascontiguousarray(
        k8.reshape(DI, C, D).transpose(2, 1, 0))
    kkb = np.ascontiguousarray(
        kernel.reshape(ICH, P, C * D).transpose(1, 0, 2)).astype(bf16)
    in_maps = []
    for s in range(NCORES):
        xc = x[s * NB:(s + 1) * NB]                      # [NB, N, DI]
        x_in = np.ascontiguousarray(
            xc.reshape(NB, NCH, P, DI).transpose(2, 0, 1, 3)).astype(bf16)
        xT_in = np.ascontiguousarray(
            xc.reshape(NB, N, ICH, P).transpose(3, 0, 2, 1)).astype(e3m4)
        xb = xc.sum(axis=1) / C                          # [NB, DI] fp32
        xbar_in = np.ascontiguousarray(
            xb.reshape(NB, ICH, P).transpose(2, 1, 0)).astype(bf16)
        in_maps.append(
            {"x": x_in, "xT": xT_in, "kk": kk, "kt": kt, "kkb": kkb,
             "xbar": xbar_in}
        )
    return in_maps


def kernel(x, kernel, _trace=False, _reps=0, _no_dma=False):
    from concourse.bass_utils import run_bass_kernel_spmd

    x = np.ascontiguousarray(np.asarray(x, dtype=np.float32))
    kernel = np.ascontiguousarray(np.asarray(kernel, dtype=np.float32))
    assert x.shape == (B, N, DI) and kernel.shape == (DI, C * D)

    key = ("nc", _reps, _no_dma)
    if key not in _cache:
        _cache[key] = _build_program(reps=_reps, no_dma=_no_dma)
    nc = _cache[key]

    in_maps = _prep_inputs(x, kernel)
    res = run_bass_kernel_spmd(nc, in_maps, list(range(NCORES)), trace=_trace)
    _cache["last_result"] = res

    out = np.empty((B, C, D), dtype=np.float32)
    for s in range(NCORES):
        o = res.results[s]["out"]                        # [d, (c,b)]
        out[s * NB:(s + 1) * NB] = o.reshape(D, C, NB).transpose(2, 1, 0)
    return out

